# revision 47
# baseline (speedup 1.0000x reference)
"""Bass/Trainium2 kernel for conv-QKV multi-head attention.

Problem: x (2,5,640,32,32); 3x3 SAME conv projections Q/K/V (640->640);
8-head attention over N=1024 tokens per (b,m) crop, head_dim=80; output
projection (640x640) applied per (b,n,m); output (2,1024,3200).

Sharding: tensor-parallel by head. Core h computes the 240 conv output
channels for head h's q/k/v (channel order [q,k,v], two tiles of
128+112 rows), full attention for its head over all 10 crops, and a
partial output projection against w_proj[:, h*80:(h+1)*80]. The 8
partial outputs are summed on the host.

Conv: 2-D Winograd F(2x2, 3x3) (points {0,1,-1,inf} per dim): 16
products per 2x2 output tile = 4/9 of direct-conv MACs. The host
pre-computes U = B^T d B per 4x4 input patch (16 j-planes of 16x16
tiles, exact cover of the 32x32 image) and Wg = G g G^T, both in fp16
(same PE rate as bf16, 8x finer mantissa -> total rel err ~6e-3 vs
1.8e-2 for the old bf16 1-D F(3,3) scheme). On device the PE
accumulates Y[jH,jW] over the 5 input-channel tiles in 16 j-planes x
2 co-tiles of 256-col matmuls (256 cols ~ 109ns stream > ~100ns
LDWEIGHTS, so weight loads stay hidden). PSUM: 4 banks hold Y[jW]
with the two co-tiles packed side by side ([*,0:256]/[*,256:512]);
per jH-group, stage-1 (A^T along jW, one ACT staging copy + DVE adds,
bias folded into the T0/T3 terms) drains each co half into fp16 SBUF
T tiles while the other co half accumulates; stage-2 (A^T along jH)
runs on the otherwise-idle GPSIMD engine, writing the fp16 qkv slabs.
Output pixel order per crop: slab(oW,oH) of 16x16 tile cols; the host
unpermutes.

Attention (per head, S^T layout so softmax-sum is a matmul row): S^T
matmuls (fp16 q/k) and exp are split at 512-col granularity so PSUM
banks recycle at the rate ACT drains them; V is transposed on the PE
into a single 1-bank PSUM staging tile and copied to the [V^T;1]
operand with one DVE op. The O matmul's ones-column emits the softmax
row-sum; the division happens on the host. P/O/out stay bf16 for the
unnormalized-exp range. The last crop runs a merged epilogue: its S^T
units, proj(8) units and the O matmul's kb-steps interleave (per-kb vT
copies), the final kb-step is h2-split so proj(9) chases the first ot
half, staging on DVE with per-half out DMAs.
"""

import numpy as np
import ml_dtypes
from contextlib import ExitStack

BS, MC, C, H, W = 2, 5, 640, 32, 32
NH, HD = 8, 80
N = H * W           # 1024
CROPS = BS * MC     # 10
CIT = C // 128      # 5 input-channel tiles
SCALE = HD ** -0.5
NCORES = 8
VB = 97             # V^T block: 80 v-dims, 16 zero, 1 ones (row sums)

_BF16 = ml_dtypes.bfloat16
# F(2,3) Toom-Cook, points {0, 1, -1, inf}: y = A^T [(G g) * (B^T d)]
_AT = np.array([[1., 1., 1., 0.], [0., 1., -1., 1.]])
_G = np.array([[-1., 0., 0.], [.5, .5, .5], [.5, -.5, .5], [0., 0., 1.]])
_BT = np.array([[-1., 0., 1., 0.], [0., 1., 1., 0.],
                [0., -1., 1., 0.], [0., -1., 0., 1.]])
NJ = 4              # Winograd points per dim
TT = 16             # tiles per dim (16 x 2 = 32, exact)
TS = TT * TT        # 256 spatial tile columns per j-plane
GCOLS = CIT * NJ * TS   # 5120 cols per jH-group in the U buffer


def _build_graph():
    import concourse.bacc as bacc
    from concourse import bass, mybir, tile, masks

    f32 = mybir.dt.float32
    bf16 = mybir.dt.bfloat16
    fp16 = mybir.dt.float16
    Exp = mybir.ActivationFunctionType.Exp
    Ident = mybir.ActivationFunctionType.Identity
    ADD = mybir.AluOpType.add
    SUB = mybir.AluOpType.subtract

    nc = bacc.Bacc("TRN2", target_bir_lowering=False, debug=False,
                   num_devices=NCORES)

    u_ext = nc.declare_dram_parameter("u", [CROPS, 128, NJ * GCOLS], fp16, isOutput=False)
    wg_ext = nc.declare_dram_parameter("wg", [CIT, 128, NJ * NJ * 256], fp16, isOutput=False)
    bqkv_ext = nc.declare_dram_parameter("bqkv", [256, 1], f32, isOutput=False)
    wproj_ext = nc.declare_dram_parameter("wproj", [HD, C], bf16, isOutput=False)
    out_ext = nc.declare_dram_parameter("out", [CROPS, C, N], bf16, isOutput=True)
    rsum_ext = nc.declare_dram_parameter("rsum", [CROPS, 1, N], f32, isOutput=True)

    with tile.TileContext(nc) as tc, ExitStack() as ctx:
        const = ctx.enter_context(tc.tile_pool(name="const", bufs=1))
        sb = ctx.enter_context(tc.tile_pool(name="sb", bufs=2))
        psum = ctx.enter_context(tc.tile_pool(name="psum", bufs=2, space="PSUM"))

        ident = const.tile([128, 128], fp16, tag="ident")
        masks.make_identity(nc, ident[:])

        # Winograd weights, one tile (and one DMA chunk per jH) per ci tile.
        wg_sb = [const.tile([128, NJ * NJ * 256], fp16, tag=f"wg{t}", name=f"wg{t}")
                 for t in range(CIT)]
        bias_a = const.tile([128, 1], f32, tag="bias_a")
        nc.sync.dma_start(bias_a[:], bqkv_ext[0:128])
        bias_b = const.tile([128, 1], f32, tag="bias_b")
        nc.sync.dma_start(bias_b[:], bqkv_ext[128:256])
        wp_sb = const.tile([HD, C], bf16, tag="wproj")
        nc.sync.dma_start(wp_sb[:], wproj_ext[:])

        # Double-buffered Winograd input planes U[jH,jW] (B^T d B on the
        # host). Layout [128, jH(4), ci(5), jW(4), 256].
        ubig = [const.tile([128, NJ * GCOLS], fp16, tag=f"u{s}", name=f"u{s}")
                for s in range(2)]
        pT = [const.tile([128, N], bf16, tag=f"pT{kb}", name=f"pT{kb}")
              for kb in range(8)]
        # vT blocks of 97 columns: [0:80] = v^T, [80:96] zero, col 96 = ones
        # (the O matmul then also emits the softmax row-sum as output row 96).
        vT2 = const.tile([128, 8 * VB], bf16, tag="vt", name="vt")
        nc.vector.memset(vT2[:], 0.0)
        for kb in range(8):
            nc.vector.memset(vT2[:, kb * VB + 96: kb * VB + VB], 1.0)

        def xload(c):
            nc.sync.dma_start(ubig[c % 2][:], u_ext[c])

        qkv_of = {}

        def conv_phase(c, fillers=()):
            """F(2x2,3x3) Winograd conv: 4 jH-groups x [2 co x 20 matmuls
            of 256 cols]; the 4 Y[jW] banks hold both co halves packed.
            Stage-1 (A^T over jW) drains each co half into SBUF T tiles
            while the other half accumulates; stage-2 (A^T over jH) runs
            on GPSIMD after the last group, writing the bf16 qkv slabs
            with bias folded in. Filler units (previous crop's attention
            front half) are spliced between ci-runs; `mid` (the 2-crops-
            ago projection) runs between groups 1 and 2."""
            su = c % 2
            qkv_a = sb.tile([128, N], fp16, tag="qkv_a")
            qkv_b = sb.tile([128, N], fp16, tag="qkv_b")
            qkv_of[c] = (qkv_a, qkv_b)
            fillers = list(fillers)
            skip = [2]  # delay first fillers: qkv(c) isn't fully written
                        # (stage-2 tail) until ~3us into this conv phase

            def pop():
                if skip[0] > 0:
                    skip[0] -= 1
                elif fillers:
                    fillers.pop(0)()

            T = {}

            def stage2(co):
                # A^T over jH on GPSIMD: out(oH=0) = T0+T1+T2,
                # out(oH=1) = T1-T2+T3; qkv col layout:
                # slab(oW,oH)*256 + t2*16 + t1.
                q = qkv_a if co == 0 else qkv_b
                for ow in range(2):
                    t = sb.tile([128, 256], f32, tag="g0")
                    nc.gpsimd.tensor_add(out=t[:], in0=T[(ow, 0, co)][:],
                                         in1=T[(ow, 1, co)][:])
                    sl = (ow * 2 + 0) * 256
                    nc.gpsimd.tensor_add(out=q[:, sl:sl + 256], in0=t[:],
                                         in1=T[(ow, 2, co)][:])
                    t2 = sb.tile([128, 256], f32, tag="g1")
                    nc.gpsimd.tensor_sub(out=t2[:], in0=T[(ow, 1, co)][:],
                                         in1=T[(ow, 2, co)][:])
                    sl = (ow * 2 + 1) * 256
                    nc.gpsimd.tensor_add(out=q[:, sl:sl + 256], in0=t2[:],
                                         in1=T[(ow, 3, co)][:])

            for g in range(NJ):
                ys = [psum.tile([128, 512], f32, tag=f"Y{jw}", bufs=1,
                                name=f"Y{jw}")
                      for jw in range(NJ)]
                for co in range(2):
                    half = slice(co * 256, co * 256 + 256)
                    for ci in range(CIT):
                        for jw in range(NJ):
                            uo = g * GCOLS + (ci * NJ + jw) * TS
                            wo = ((g * NJ + jw) * 2 + co) * 128
                            nc.tensor.matmul(ys[jw][:, half],
                                             wg_sb[ci][:, wo:wo + 128],
                                             ubig[su][:, uo:uo + TS],
                                             start=(ci == 0), stop=(ci == CIT - 1))
                        if ci == 2:
                            pop()
                    # stage-1: T0 = Y0+Y1+Y2, T1 = Y1-Y2+Y3 for this co half
                    # (single ACT staging of Y1; each DVE op reads <=1 PSUM)
                    a = sb.tile([128, 256], f32, tag="sa")
                    nc.scalar.activation(a[:], ys[1][:, half], Ident)
                    b = sb.tile([128, 256], f32, tag="sb1")
                    nc.vector.tensor_add(out=b[:], in0=a[:], in1=ys[2][:, half])
                    bias = bias_a if co == 0 else bias_b
                    t0 = sb.tile([128, 256], fp16, tag=f"T0_{g}_{co}", bufs=1,
                                 name=f"T0_{g}_{co}")
                    if g == 0:
                        # bias folded into T0: it only feeds the oH=0 output
                        nc.vector.scalar_tensor_tensor(
                            out=t0[:], in0=b[:], scalar=bias[:],
                            in1=ys[0][:, half], op0=ADD, op1=ADD)
                    else:
                        nc.vector.tensor_add(out=t0[:], in0=b[:],
                                             in1=ys[0][:, half])
                    c2 = sb.tile([128, 256], f32, tag="sc")
                    nc.vector.tensor_sub(out=c2[:], in0=a[:], in1=ys[2][:, half])
                    t1 = sb.tile([128, 256], fp16, tag=f"T1_{g}_{co}", bufs=1,
                                 name=f"T1_{g}_{co}")
                    if g == 3:
                        # bias folded into T3: it only feeds the oH=1 output
                        nc.vector.scalar_tensor_tensor(
                            out=t1[:], in0=c2[:], scalar=bias[:],
                            in1=ys[3][:, half], op0=ADD, op1=ADD)
                    else:
                        nc.vector.tensor_add(out=t1[:], in0=c2[:],
                                             in1=ys[3][:, half])
                    T[(0, g, co)] = t0
                    T[(1, g, co)] = t1
                    if g == 3:
                        stage2(co)
                    pop()
            for f in fillers:
                f()

        ot_of = {}

        def attn_fillers(c, split_vt=False):
            """Per-kb PE units of crop c's attention front half: vT
            transpose into a 1-bank PSUM staging tile + S^T matmuls with
            per-512-col exp. Final unit copies the staged V^T out (or,
            with split_vt, each unit copies its own block so the O
            matmul's kb-steps can chase the units)."""
            qkv_a, qkv_b = qkv_of[c]
            v_sb = sb.tile([HD, N], fp16, tag="v_sb", bufs=1)
            nc.sync.dma_start(v_sb[0:48, :], qkv_a[80:128, :])
            nc.sync.dma_start(v_sb[48:80, :], qkv_b[80:112, :])
            # staging stride 98 (not 97) so each bf16 block is 4B-aligned
            TVB = 98
            tva = psum.tile([128, 8 * TVB], fp16, tag="tv", bufs=1)

            def unit(kb):
                def f():
                    nc.tensor.transpose(tva[:, kb * TVB: kb * TVB + HD],
                                        v_sb[:, kb * 128:(kb + 1) * 128],
                                        ident[0:HD, 0:HD])
                    for h2 in range(2):
                        st = psum.tile([128, 512], f32, tag="st", bufs=3)
                        nc.tensor.matmul(
                            st[:],
                            qkv_b[0:HD, kb * 128:(kb + 1) * 128],
                            qkv_a[0:HD, h2 * 512:(h2 + 1) * 512],
                            start=True, stop=True)
                        nc.scalar.activation(
                            pT[kb][:, h2 * 512:(h2 + 1) * 512], st[:],
                            Exp, scale=SCALE)
                    if split_vt:
                        nc.vector.tensor_copy(
                            vT2[:, kb * VB: kb * VB + HD],
                            tva[:, kb * TVB: kb * TVB + HD])
                return f

            def vt_copy():
                dst = vT2[:].rearrange("p (kb vb) -> p kb vb", vb=VB)[:, :, 0:HD]
                src = tva[:].rearrange("p (kb vb) -> p kb vb", vb=TVB)[:, :, 0:HD]
                nc.vector.tensor_copy(dst, src)

            units = [unit(kb) for kb in range(8)]
            return units if split_vt else units + [vt_copy]

        def attn_tail(c):
            """O matmul + output staging for crop c (after its fillers).
            [O^T; rowsum] = [V^T; 1]^T P^T  (row 96 = softmax sums)."""
            qkv_of.pop(c)
            ot = sb.tile([HD, N], bf16, tag="ot")
            rrow = sb.tile([1, N], f32, tag="rrow")
            # kb-outer / h2-inner: consecutive matmuls share the vt weight
            # block and alternate the two accumulation banks
            po = [psum.tile([VB, 512], f32, tag="st", name=f"po{h2}", bufs=3)
                  for h2 in range(2)]
            for kb in range(8):
                for h2 in range(2):
                    nc.tensor.matmul(
                        po[h2][:], vT2[:, kb * VB:(kb + 1) * VB],
                        pT[kb][:, h2 * 512:(h2 + 1) * 512],
                        start=(kb == 0), stop=(kb == 7))
            for h2 in range(2):
                nc.scalar.activation(ot[:, h2 * 512:(h2 + 1) * 512],
                                     po[h2][0:HD, :], Ident)
                nc.vector.tensor_copy(rrow[:, h2 * 512:(h2 + 1) * 512],
                                      po[h2][96:97, :])
            ot_of[c] = ot
            nc.sync.dma_start(rsum_ext[c], rrow[:])

        def proj_units(c):
            ot = ot_of.pop(c)
            def u(dt):
                def f():
                    osb = sb.tile([128, N], bf16, tag="osb")
                    for h2 in range(2):
                        pp = psum.tile([128, 512], f32, tag="st", bufs=3)
                        nc.tensor.matmul(pp[:], wp_sb[:, dt * 128:(dt + 1) * 128],
                                         ot[:, h2 * 512:(h2 + 1) * 512],
                                         start=True, stop=True)
                        if h2 == 0:
                            nc.scalar.activation(osb[:, 0:512], pp[:], Ident)
                        else:
                            nc.vector.tensor_copy(osb[:, 512:1024], pp[:])
                    nc.sync.dma_start(out_ext[c, dt * 128:(dt + 1) * 128, :],
                                      osb[:])
                return f
            return [u(dt) for dt in range(5)]

        def proj_phase(c, fillers=()):
            fillers = list(fillers)
            for u in proj_units(c):
                u()
                for _ in range(2):
                    if fillers:
                        fillers.pop(0)()
            for f in fillers:
                f()

        def epilogue(c):
            """Last crop: interleave its S^T units, proj(c-1) units and the
            O matmul's kb-steps (per-kb vT copies let O chase the units),
            then drain and run proj(c) immediately."""
            fills = attn_fillers(c, split_vt=True)
            pd = proj_units(c - 1)
            qkv_of.pop(c)
            # po on the (now idle) conv Y banks, freeing the st rotation
            # for the S units
            po = [psum.tile([VB, 512], f32, tag=f"Y{h2}", bufs=1,
                            name=f"po{h2}")
                  for h2 in range(2)]

            def osteph(h2, kb):
                def f():
                    nc.tensor.matmul(
                        po[h2][:], vT2[:, kb * VB:(kb + 1) * VB],
                        pT[kb][:, h2 * 512:(h2 + 1) * 512],
                        start=(kb == 0), stop=(kb == 7))
                return f

            ot = sb.tile([HD, N], bf16, tag="ot")
            rrow = sb.tile([1, N], f32, tag="rrow")

            def projh(h2, dt):
                def f():
                    pp = psum.tile([128, 512], f32, tag="st", bufs=3)
                    nc.tensor.matmul(pp[:], wp_sb[:, dt * 128:(dt + 1) * 128],
                                     ot[:, h2 * 512:(h2 + 1) * 512],
                                     start=True, stop=True)
                    osb = sb.tile([128, 512], bf16, tag=f"osbh{dt % 2}",
                                  name=f"osbh{dt % 2}")
                    # staging chains split: h2=0 on DVE, h2=1 on ACT
                    # (after ot1's activation, so ot1 is never delayed)
                    if h2 == 0:
                        nc.vector.tensor_copy(osb[:], pp[:])
                    else:
                        nc.scalar.activation(osb[:], pp[:], Ident)
                    nc.sync.dma_start(
                        out_ext[c, dt * 128:(dt + 1) * 128,
                                h2 * 512:(h2 + 1) * 512], osb[:])
                return f

            # h2-major O: the h2=0 half finishes 8 steps early, so its ot
            # half stages while the h2=1 steps run, and proj-h0 interleaves
            # with them (also breaking po[1]'s same-bank accumulation runs)
            seq = [fills[0], fills[1], pd[0], osteph(0, 0),
                   fills[2], pd[1], osteph(0, 1),
                   fills[3], pd[2], osteph(0, 2),
                   fills[4], pd[3], osteph(0, 3),
                   fills[5], pd[4], osteph(0, 4),
                   fills[6], osteph(0, 5),
                   fills[7], osteph(0, 6), osteph(0, 7)]
            for f in seq:
                f()
            nc.scalar.activation(ot[:, 0:512], po[0][0:HD, :], Ident)
            nc.vector.tensor_copy(rrow[:, 0:512], po[0][96:97, :])
            seq2 = [osteph(1, 0), osteph(1, 1), projh(0, 0),
                    osteph(1, 2), projh(0, 1), osteph(1, 3), projh(0, 2),
                    osteph(1, 4), projh(0, 3), osteph(1, 5), osteph(1, 6),
                    projh(0, 4), osteph(1, 7)]
            for f in seq2:
                f()
            nc.scalar.activation(ot[:, 512:1024], po[1][0:HD, :], Ident)
            nc.vector.tensor_copy(rrow[:, 512:1024], po[1][96:97, :])
            nc.sync.dma_start(rsum_ext[c], rrow[:])
            for dt in range(5):
                projh(1, dt)()

        # Software-pipelined emission: crop c's S^T/vT units are spliced
        # into crop c+1's conv stream (their ACT/DVE consumers overlap the
        # conv matmuls), then O(c) and proj(c-1) follow.
        # Prologue DMAs interleaved per jH-group so conv(0)'s first group
        # can start as soon as its own weight/U chunks land.
        # Prologue interleaved per jH-group: conv(0)'s group g can start
        # as soon as its own wg+u chunks land (DMA-paced, ~8us/group).
        for g in range(NJ):
            for t in range(CIT):
                nc.sync.dma_start(wg_sb[t][:, g * 1024:(g + 1) * 1024],
                                  wg_ext[t][:, g * 1024:(g + 1) * 1024])
            nc.sync.dma_start(ubig[0][:, g * GCOLS:(g + 1) * GCOLS],
                              u_ext[0][:, g * GCOLS:(g + 1) * GCOLS])
        # PE warmup: dummy matmuls on the identity while the first crop
        # loads, so conv(0) starts at full clock. (xload(1) is emitted
        # after conv(0) so its 5MB doesn't contend with crop 0's chunks.)
        warm = psum.tile([128, 128], f32, tag="st", bufs=3)
        for _ in range(175):
            nc.tensor.matmul(warm[:], ident[:], ident[:], start=True, stop=True)

        def warm_unit():
            # keep the PE streaming (and the clock up) while the early
            # crops are DMA-paced, instead of idling into the throttle
            for _ in range(6):
                nc.tensor.matmul(warm[:], ident[:], ident[:],
                                 start=True, stop=True)

        conv_phase(0, fillers=[warm_unit] * 12)
        xload(1)
        for c in range(CROPS):
            if c + 1 < CROPS:
                # fillers first: their v_sb DMAs sit AHEAD of the big
                # xload in the FIFO queue, so the early transpose units
                # aren't starved behind a 5MB transfer + its WAR wait
                fills = attn_fillers(c)
                if c == 0:
                    fills = fills + [warm_unit] * 5
                if c >= 1:
                    # proj(c-1) spread across filler slots instead of one
                    # mid-conv burst: evens out the ACT/DVE staging load
                    fills = fills + proj_units(c - 1)
                if c + 2 < CROPS:
                    xload(c + 2)
                conv_phase(c + 1, fills)
                attn_tail(c)
            else:
                epilogue(c)

    nc.compile()
    return nc


def _host_inputs(x, wq, bq, wk, bk, wv, bv, w_proj):
    """Per-core input maps; conv output channels ordered [q, k, v]."""
    # Shared across cores: the 2-D Winograd F(2x2,3x3) input transform
    # U = B^T d B per 4x4 patch (stride 2), computed on the host from the
    # bf16-cast padded image. SBUF layout [128, jH, ci, jW, t2*16+t1].
    xf = np.asarray(x, dtype=np.float32).reshape(CROPS, C, H, W)
    xpad = np.zeros((CROPS, C, H + 2, W + 2), np.float32)
    xpad[:, :, 1:1 + H, 1:1 + W] = xf.astype(_BF16).astype(np.float32)
    win = np.lib.stride_tricks.sliding_window_view(xpad, (NJ, NJ), axis=(2, 3))
    win = win[:, :, ::2, ::2]                     # [n, c, 16, 16, a2, a1]
    u = np.einsum('ja,kb,nctsab->ncjkts', _BT, _BT, win, optimize=True)
    u = u.reshape(CROPS, CIT, 128, NJ, NJ, TS).transpose(0, 2, 3, 1, 4, 5)
    u = np.ascontiguousarray(u).reshape(CROPS, 128, NJ * GCOLS).astype(np.float16)

    in_maps = []
    for h in range(NCORES):
        sl = slice(h * HD, (h + 1) * HD)
        zpad = np.zeros((16,) + wq.shape[1:], wq.dtype)
        w_cat = np.concatenate(
            [wq[sl], wv[sl][:48], wk[sl], wv[sl][48:], zpad], axis=0)  # [256,...]
        # 2-D G-transform: [CIT, 128, jH, jW, co] = [5, 128, 4096]
        wg = np.einsum('jd,ke,ocde->cjko', _G, _G, w_cat.astype(np.float64),
                       optimize=True)
        wg = wg.reshape(CIT, 128, NJ, NJ, 256).reshape(CIT, 128, NJ * NJ * 256)
        b_cat = np.concatenate(
            [bq[sl], bv[sl][:48], bk[sl], bv[sl][48:],
             np.zeros(16, bq.dtype)]).reshape(256, 1)
        wpT = np.ascontiguousarray(w_proj[:, sl].T)  # [80, 640]
        in_maps.append({
            "u": u,
            "wg": np.ascontiguousarray(wg).astype(np.float16),
            "bqkv": b_cat.astype(np.float32),
            "wproj": wpT.astype(_BF16),
        })
    return in_maps


def _host_reduce(results, b_proj):
    acc = np.zeros((CROPS, C, N), np.float32)
    for r in results:
        acc += r["out"].astype(np.float32) / r["rsum"]
    # un-permute n' = slab(oW,oH)*256 + t2*16 + t1 back to n = y*32 + x
    idx = np.empty(N, np.int64)
    pos = 0
    for ow in range(2):
        for oh in range(2):
            for t2 in range(TT):
                for t1 in range(TT):
                    idx[pos] = (2 * t2 + oh) * W + (2 * t1 + ow)
                    pos += 1
    out = np.empty_like(acc)
    out[:, :, idx] = acc
    acc = out
    o = acc.reshape(BS, MC, C, N).transpose(0, 3, 1, 2)  # [b, n, m, dout]
    o = o + b_proj[None, None, None, :].astype(np.float32)
    return np.ascontiguousarray(o.reshape(BS, N, MC * C), dtype=np.float32)


_NC_CACHE = {}


def kernel(x, wq, bq, wk, bk, wv, bv, w_proj, b_proj, _run_kwargs=None):
    from concourse.bass_utils import run_bass_kernel_spmd

    if "nc" not in _NC_CACHE:
        _NC_CACHE["nc"] = _build_graph()
    nc = _NC_CACHE["nc"]
    in_maps = _host_inputs(x, wq, bq, wk, bk, wv, bv, w_proj)
    res = run_bass_kernel_spmd(nc, in_maps, core_ids=list(range(NCORES)),
                               **(_run_kwargs or {}))
    out = _host_reduce(res.results, np.asarray(b_proj))
    if _run_kwargs:
        _NC_CACHE["last_result"] = res
    return out


# revision 48
# speedup vs baseline: 1.1786x; 1.1786x over previous
"""Bass/Trainium2 kernel for conv-QKV multi-head attention.

Problem: x (2,5,640,32,32); 3x3 SAME conv projections Q/K/V (640->640);
8-head attention over N=1024 tokens per (b,m) crop, head_dim=80; output
projection (640x640) applied per (b,n,m); output (2,1024,3200).

Sharding: tensor-parallel by head. Core h computes the 240 conv output
channels for head h's q/k/v (channel order [q,k,v], two tiles of
128+112 rows), full attention for its head over all 10 crops, and a
partial output projection against w_proj[:, h*80:(h+1)*80]. The 8
partial outputs are summed on the host.

Conv: 2-D Winograd F(2x2, 3x3) (points {0,1,-1,inf} per dim): 16
products per 2x2 output tile = 4/9 of direct-conv MACs. The host
pre-computes U = B^T d B per 4x4 input patch (16 j-planes of 16x16
tiles, exact cover of the 32x32 image) and Wg = G g G^T, both in fp16
(same PE rate as bf16, 8x finer mantissa -> total rel err ~6e-3 vs
1.8e-2 for the old bf16 1-D F(3,3) scheme). On device the PE
accumulates Y[jH,jW] over the 5 input-channel tiles in 16 j-planes x
2 co-tiles of 256-col matmuls (256 cols ~ 109ns stream > ~100ns
LDWEIGHTS, so weight loads stay hidden). PSUM: 4 banks hold Y[jW]
with the two co-tiles packed side by side ([*,0:256]/[*,256:512]);
per jH-group, stage-1 (A^T along jW, one ACT staging copy + DVE adds,
bias folded into the T0/T3 terms) drains each co half into fp16 SBUF
T tiles while the other co half accumulates; stage-2 (A^T along jH)
runs on the otherwise-idle GPSIMD engine, writing the fp16 qkv slabs.
Output pixel order per crop: slab(oW,oH) of 16x16 tile cols; the host
unpermutes.

Attention (per head, S^T layout so softmax-sum is a matmul row): S^T
matmuls (fp16 q/k) and exp are split at 512-col granularity so PSUM
banks recycle at the rate ACT drains them; V is transposed on the PE
into a single 1-bank PSUM staging tile and copied to the [V^T;1]
operand with one DVE op. The O matmul's ones-column emits the softmax
row-sum; the division happens on the host. P/O/out stay bf16 for the
unnormalized-exp range. The last crop runs a merged epilogue: its S^T
units, proj(8) units and the O matmul's kb-steps interleave (per-kb vT
copies), the final kb-step is h2-split so proj(9) chases the first ot
half, staging on DVE with per-half out DMAs.
"""

import numpy as np
import ml_dtypes
from contextlib import ExitStack

BS, MC, C, H, W = 2, 5, 640, 32, 32
NH, HD = 8, 80
N = H * W           # 1024
CROPS = BS * MC     # 10
CIT = C // 128      # 5 input-channel tiles
SCALE = HD ** -0.5
NCORES = 8
VB = 97             # V^T block: 80 v-dims, 16 zero, 1 ones (row sums)

_BF16 = ml_dtypes.bfloat16
# F(2,3) Toom-Cook, points {0, 1, -1, inf}: y = A^T [(G g) * (B^T d)]
_AT = np.array([[1., 1., 1., 0.], [0., 1., -1., 1.]])
_G = np.array([[-1., 0., 0.], [.5, .5, .5], [.5, -.5, .5], [0., 0., 1.]])
_BT = np.array([[-1., 0., 1., 0.], [0., 1., 1., 0.],
                [0., -1., 1., 0.], [0., -1., 0., 1.]])
NJ = 4              # Winograd points per dim
TT = 16             # tiles per dim (16 x 2 = 32, exact)
TS = TT * TT        # 256 spatial tile columns per j-plane
GCOLS = CIT * NJ * TS   # 5120 cols per jH-group in the U buffer


def _build_graph():
    import concourse.bacc as bacc
    from concourse import bass, mybir, tile, masks

    f32 = mybir.dt.float32
    bf16 = mybir.dt.bfloat16
    fp16 = mybir.dt.float16
    Exp = mybir.ActivationFunctionType.Exp
    Ident = mybir.ActivationFunctionType.Identity
    ADD = mybir.AluOpType.add
    SUB = mybir.AluOpType.subtract

    nc = bacc.Bacc("TRN2", target_bir_lowering=False, debug=False,
                   num_devices=NCORES)

    u_ext = nc.declare_dram_parameter("u", [CROPS, 128, NJ * GCOLS], fp16, isOutput=False)
    wg_ext = nc.declare_dram_parameter("wg", [CIT, 128, NJ * NJ * 256], fp16, isOutput=False)
    bqkv_ext = nc.declare_dram_parameter("bqkv", [256, 1], f32, isOutput=False)
    wproj_ext = nc.declare_dram_parameter("wproj", [HD, C], bf16, isOutput=False)
    out_ext = nc.declare_dram_parameter("out", [CROPS, C, N], bf16, isOutput=True)
    rsum_ext = nc.declare_dram_parameter("rsum", [CROPS, 1, N], f32, isOutput=True)

    with tile.TileContext(nc) as tc, ExitStack() as ctx:
        const = ctx.enter_context(tc.tile_pool(name="const", bufs=1))
        sb = ctx.enter_context(tc.tile_pool(name="sb", bufs=2))
        psum = ctx.enter_context(tc.tile_pool(name="psum", bufs=2, space="PSUM"))

        ident = const.tile([128, 128], fp16, tag="ident")
        masks.make_identity(nc, ident[:])

        # Winograd weights, one tile (and one DMA chunk per jH) per ci tile.
        wg_sb = [const.tile([128, NJ * NJ * 256], fp16, tag=f"wg{t}", name=f"wg{t}")
                 for t in range(CIT)]
        bias_a = const.tile([128, 1], f32, tag="bias_a")
        nc.sync.dma_start(bias_a[:], bqkv_ext[0:128])
        bias_b = const.tile([128, 1], f32, tag="bias_b")
        nc.sync.dma_start(bias_b[:], bqkv_ext[128:256])
        wp_sb = const.tile([HD, C], bf16, tag="wproj")
        nc.sync.dma_start(wp_sb[:], wproj_ext[:])

        # Double-buffered Winograd input planes U[jH,jW] (B^T d B on the
        # host). Layout [128, jH(4), ci(5), jW(4), 256].
        ubig = [const.tile([128, NJ * GCOLS], fp16, tag=f"u{s}", name=f"u{s}")
                for s in range(2)]
        pT = [const.tile([128, N], bf16, tag=f"pT{kb}", name=f"pT{kb}")
              for kb in range(8)]
        # vT blocks of 97 columns: [0:80] = v^T, [80:96] zero, col 96 = ones
        # (the O matmul then also emits the softmax row-sum as output row 96).
        vT2 = const.tile([128, 8 * VB], bf16, tag="vt", name="vt")
        nc.vector.memset(vT2[:], 0.0)
        for kb in range(8):
            nc.vector.memset(vT2[:, kb * VB + 96: kb * VB + VB], 1.0)

        def xload(c):
            nc.sync.dma_start(ubig[c % 2][:], u_ext[c])

        qkv_of = {}

        def conv_phase(c, fillers=()):
            """F(2x2,3x3) Winograd conv: 4 jH-groups x [2 co x 20 matmuls
            of 256 cols]; the 4 Y[jW] banks hold both co halves packed.
            Stage-1 (A^T over jW) drains each co half into SBUF T tiles
            while the other half accumulates; stage-2 (A^T over jH) runs
            on GPSIMD after the last group, writing the bf16 qkv slabs
            with bias folded in. Filler units (previous crop's attention
            front half) are spliced between ci-runs; `mid` (the 2-crops-
            ago projection) runs between groups 1 and 2."""
            su = c % 2
            qkv_a = sb.tile([128, N], fp16, tag="qkv_a")
            qkv_b = sb.tile([128, N], fp16, tag="qkv_b")
            qkv_of[c] = (qkv_a, qkv_b)
            fillers = list(fillers)
            skip = [2]  # delay first fillers: qkv(c) isn't fully written
                        # (stage-2 tail) until ~3us into this conv phase

            def pop():
                if skip[0] > 0:
                    skip[0] -= 1
                elif fillers:
                    fillers.pop(0)()

            T = {}

            def stage2(co):
                # A^T over jH on GPSIMD: out(oH=0) = T0+T1+T2,
                # out(oH=1) = T1-T2+T3; qkv col layout:
                # slab(oW,oH)*256 + t2*16 + t1.
                q = qkv_a if co == 0 else qkv_b
                for ow in range(2):
                    t = sb.tile([128, 256], f32, tag="g0")
                    nc.gpsimd.tensor_add(out=t[:], in0=T[(ow, 0, co)][:],
                                         in1=T[(ow, 1, co)][:])
                    sl = (ow * 2 + 0) * 256
                    nc.gpsimd.tensor_add(out=q[:, sl:sl + 256], in0=t[:],
                                         in1=T[(ow, 2, co)][:])
                    t2 = sb.tile([128, 256], f32, tag="g1")
                    nc.gpsimd.tensor_sub(out=t2[:], in0=T[(ow, 1, co)][:],
                                         in1=T[(ow, 2, co)][:])
                    sl = (ow * 2 + 1) * 256
                    nc.gpsimd.tensor_add(out=q[:, sl:sl + 256], in0=t2[:],
                                         in1=T[(ow, 3, co)][:])

            for g in range(NJ):
                ys = [psum.tile([128, 512], f32, tag=f"Y{jw}", bufs=1,
                                name=f"Y{jw}")
                      for jw in range(NJ)]
                for co in range(2):
                    half = slice(co * 256, co * 256 + 256)
                    for ci in range(CIT):
                        for jw in range(NJ):
                            uo = g * GCOLS + (ci * NJ + jw) * TS
                            wo = ((g * NJ + jw) * 2 + co) * 128
                            nc.tensor.matmul(ys[jw][:, half],
                                             wg_sb[ci][:, wo:wo + 128],
                                             ubig[su][:, uo:uo + TS],
                                             start=(ci == 0), stop=(ci == CIT - 1))
                        if ci == 2:
                            pop()
                    # stage-1: T0 = Y0+Y1+Y2, T1 = Y1-Y2+Y3 for this co half
                    # (single ACT staging of Y1; each DVE op reads <=1 PSUM)
                    a = sb.tile([128, 256], f32, tag="sa")
                    nc.scalar.activation(a[:], ys[1][:, half], Ident)
                    b = sb.tile([128, 256], f32, tag="sb1")
                    nc.vector.tensor_add(out=b[:], in0=a[:], in1=ys[2][:, half])
                    bias = bias_a if co == 0 else bias_b
                    t0 = sb.tile([128, 256], fp16, tag=f"T0_{g}_{co}", bufs=1,
                                 name=f"T0_{g}_{co}")
                    if g == 0:
                        # bias folded into T0: it only feeds the oH=0 output
                        nc.vector.scalar_tensor_tensor(
                            out=t0[:], in0=b[:], scalar=bias[:],
                            in1=ys[0][:, half], op0=ADD, op1=ADD)
                    else:
                        nc.vector.tensor_add(out=t0[:], in0=b[:],
                                             in1=ys[0][:, half])
                    c2 = sb.tile([128, 256], f32, tag="sc")
                    nc.vector.tensor_sub(out=c2[:], in0=a[:], in1=ys[2][:, half])
                    t1 = sb.tile([128, 256], fp16, tag=f"T1_{g}_{co}", bufs=1,
                                 name=f"T1_{g}_{co}")
                    if g == 3:
                        # bias folded into T3: it only feeds the oH=1 output
                        nc.vector.scalar_tensor_tensor(
                            out=t1[:], in0=c2[:], scalar=bias[:],
                            in1=ys[3][:, half], op0=ADD, op1=ADD)
                    else:
                        nc.vector.tensor_add(out=t1[:], in0=c2[:],
                                             in1=ys[3][:, half])
                    T[(0, g, co)] = t0
                    T[(1, g, co)] = t1
                    if g == 3:
                        stage2(co)
                    pop()
            for f in fillers:
                f()

        ot_of = {}

        def attn_fillers(c, split_vt=False):
            """Per-kb PE units of crop c's attention front half: vT
            transpose into a 1-bank PSUM staging tile + S^T matmuls with
            per-512-col exp. Final unit copies the staged V^T out (or,
            with split_vt, each unit copies its own block so the O
            matmul's kb-steps can chase the units)."""
            qkv_a, qkv_b = qkv_of[c]
            v_sb = sb.tile([HD, N], fp16, tag="v_sb", bufs=1)
            nc.sync.dma_start(v_sb[0:48, :], qkv_a[80:128, :])
            nc.sync.dma_start(v_sb[48:80, :], qkv_b[80:112, :])
            # staging stride 98 (not 97) so each bf16 block is 4B-aligned
            TVB = 98
            tva = psum.tile([128, 8 * TVB], fp16, tag="tv", bufs=1)

            def unit(kb):
                def f():
                    nc.tensor.transpose(tva[:, kb * TVB: kb * TVB + HD],
                                        v_sb[:, kb * 128:(kb + 1) * 128],
                                        ident[0:HD, 0:HD])
                    for h2 in range(2):
                        st = psum.tile([128, 512], f32, tag="st", bufs=3)
                        nc.tensor.matmul(
                            st[:],
                            qkv_b[0:HD, kb * 128:(kb + 1) * 128],
                            qkv_a[0:HD, h2 * 512:(h2 + 1) * 512],
                            start=True, stop=True)
                        nc.scalar.activation(
                            pT[kb][:, h2 * 512:(h2 + 1) * 512], st[:],
                            Exp, scale=SCALE)
                    if split_vt:
                        nc.vector.tensor_copy(
                            vT2[:, kb * VB: kb * VB + HD],
                            tva[:, kb * TVB: kb * TVB + HD])
                return f

            def vt_copy():
                dst = vT2[:].rearrange("p (kb vb) -> p kb vb", vb=VB)[:, :, 0:HD]
                src = tva[:].rearrange("p (kb vb) -> p kb vb", vb=TVB)[:, :, 0:HD]
                nc.vector.tensor_copy(dst, src)

            units = [unit(kb) for kb in range(8)]
            return units if split_vt else units + [vt_copy]

        def attn_tail(c):
            """O matmul + output staging for crop c (after its fillers).
            [O^T; rowsum] = [V^T; 1]^T P^T  (row 96 = softmax sums)."""
            qkv_of.pop(c)
            ot = sb.tile([HD, N], bf16, tag="ot")
            rrow = sb.tile([1, N], f32, tag="rrow")
            # kb-outer / h2-inner: consecutive matmuls share the vt weight
            # block and alternate the two accumulation banks
            po = [psum.tile([VB, 512], f32, tag="st", name=f"po{h2}", bufs=3)
                  for h2 in range(2)]
            for kb in range(8):
                for h2 in range(2):
                    nc.tensor.matmul(
                        po[h2][:], vT2[:, kb * VB:(kb + 1) * VB],
                        pT[kb][:, h2 * 512:(h2 + 1) * 512],
                        start=(kb == 0), stop=(kb == 7))
            for h2 in range(2):
                nc.scalar.activation(ot[:, h2 * 512:(h2 + 1) * 512],
                                     po[h2][0:HD, :], Ident)
                nc.vector.tensor_copy(rrow[:, h2 * 512:(h2 + 1) * 512],
                                      po[h2][96:97, :])
            ot_of[c] = ot
            nc.sync.dma_start(rsum_ext[c], rrow[:])

        def proj_units(c):
            ot = ot_of.pop(c)
            def u(dt):
                def f():
                    osb = sb.tile([128, N], bf16, tag="osb")
                    for h2 in range(2):
                        pp = psum.tile([128, 512], f32, tag="st", bufs=3)
                        nc.tensor.matmul(pp[:], wp_sb[:, dt * 128:(dt + 1) * 128],
                                         ot[:, h2 * 512:(h2 + 1) * 512],
                                         start=True, stop=True)
                        if h2 == 0:
                            nc.scalar.activation(osb[:, 0:512], pp[:], Ident)
                        else:
                            nc.vector.tensor_copy(osb[:, 512:1024], pp[:])
                    nc.sync.dma_start(out_ext[c, dt * 128:(dt + 1) * 128, :],
                                      osb[:])
                return f
            return [u(dt) for dt in range(5)]

        def proj_phase(c, fillers=()):
            fillers = list(fillers)
            for u in proj_units(c):
                u()
                for _ in range(2):
                    if fillers:
                        fillers.pop(0)()
            for f in fillers:
                f()

        def epilogue(c):
            """Last crop: interleave its S^T units, proj(c-1) units and the
            O matmul's kb-steps (per-kb vT copies let O chase the units),
            then drain and run proj(c) immediately."""
            fills = attn_fillers(c, split_vt=True)
            pd = proj_units(c - 1)
            qkv_of.pop(c)
            # po on the (now idle) conv Y banks, freeing the st rotation
            # for the S units
            po = [psum.tile([VB, 512], f32, tag=f"Y{h2}", bufs=1,
                            name=f"po{h2}")
                  for h2 in range(2)]

            def osteph(h2, kb):
                def f():
                    nc.tensor.matmul(
                        po[h2][:], vT2[:, kb * VB:(kb + 1) * VB],
                        pT[kb][:, h2 * 512:(h2 + 1) * 512],
                        start=(kb == 0), stop=(kb == 7))
                return f

            ot = sb.tile([HD, N], bf16, tag="ot")
            rrow = sb.tile([1, N], f32, tag="rrow")

            def projh(h2, dt):
                def f():
                    pp = psum.tile([128, 512], f32, tag="st", bufs=3)
                    nc.tensor.matmul(pp[:], wp_sb[:, dt * 128:(dt + 1) * 128],
                                     ot[:, h2 * 512:(h2 + 1) * 512],
                                     start=True, stop=True)
                    osb = sb.tile([128, 512], bf16, tag=f"osbh{dt % 2}",
                                  name=f"osbh{dt % 2}")
                    # staging chains split: h2=0 on DVE, h2=1 on ACT
                    # (after ot1's activation, so ot1 is never delayed)
                    if h2 == 0:
                        nc.vector.tensor_copy(osb[:], pp[:])
                    else:
                        nc.scalar.activation(osb[:], pp[:], Ident)
                    nc.sync.dma_start(
                        out_ext[c, dt * 128:(dt + 1) * 128,
                                h2 * 512:(h2 + 1) * 512], osb[:])
                return f

            # h2-major O: the h2=0 half finishes 8 steps early, so its ot
            # half stages while the h2=1 steps run, and proj-h0 interleaves
            # with them (also breaking po[1]'s same-bank accumulation runs)
            seq = [fills[0], fills[1], pd[0], osteph(0, 0),
                   fills[2], pd[1], osteph(0, 1),
                   fills[3], pd[2], osteph(0, 2),
                   fills[4], pd[3], osteph(0, 3),
                   fills[5], pd[4], osteph(0, 4),
                   fills[6], osteph(0, 5),
                   fills[7], osteph(0, 6), osteph(0, 7)]
            for f in seq:
                f()
            nc.scalar.activation(ot[:, 0:512], po[0][0:HD, :], Ident)
            nc.vector.tensor_copy(rrow[:, 0:512], po[0][96:97, :])
            seq2 = [osteph(1, 0), osteph(1, 1), projh(0, 0),
                    osteph(1, 2), projh(0, 1), osteph(1, 3), projh(0, 2),
                    osteph(1, 4), projh(0, 3), osteph(1, 5), osteph(1, 6),
                    projh(0, 4), osteph(1, 7)]
            for f in seq2:
                f()
            nc.scalar.activation(ot[:, 512:1024], po[1][0:HD, :], Ident)
            nc.vector.tensor_copy(rrow[:, 512:1024], po[1][96:97, :])
            nc.sync.dma_start(rsum_ext[c], rrow[:])
            for dt in range(5):
                projh(1, dt)()

        # Software-pipelined emission: crop c's S^T/vT units are spliced
        # into crop c+1's conv stream (their ACT/DVE consumers overlap the
        # conv matmuls), then O(c) and proj(c-1) follow.
        # Prologue DMAs interleaved per jH-group so conv(0)'s first group
        # can start as soon as its own weight/U chunks land.
        # Prologue interleaved per jH-group: conv(0)'s group g can start
        # as soon as its own wg+u chunks land (DMA-paced, ~8us/group).
        for g in range(NJ):
            for t in range(CIT):
                nc.sync.dma_start(wg_sb[t][:, g * 1024:(g + 1) * 1024],
                                  wg_ext[t][:, g * 1024:(g + 1) * 1024])
            nc.sync.dma_start(ubig[0][:, g * GCOLS:(g + 1) * GCOLS],
                              u_ext[0][:, g * GCOLS:(g + 1) * GCOLS])
        # PE warmup: dummy matmuls on the identity while the first crop
        # loads, so conv(0) starts at full clock. (xload(1) is emitted
        # after conv(0) so its 5MB doesn't contend with crop 0's chunks.)
        warm = psum.tile([128, 128], f32, tag="st", bufs=3)
        for _ in range(150):
            nc.tensor.matmul(warm[:], ident[:], ident[:], start=True, stop=True)

        def warm_unit():
            # keep the PE streaming (and the clock up) while the early
            # crops are DMA-paced, instead of idling into the throttle
            for _ in range(6):
                nc.tensor.matmul(warm[:], ident[:], ident[:],
                                 start=True, stop=True)

        conv_phase(0, fillers=[warm_unit] * 12)
        xload(1)
        for c in range(CROPS):
            if c + 1 < CROPS:
                # fillers first: their v_sb DMAs sit AHEAD of the big
                # xload in the FIFO queue, so the early transpose units
                # aren't starved behind a 5MB transfer + its WAR wait
                fills = attn_fillers(c)
                if c == 0:
                    fills = fills + [warm_unit] * 5
                if c >= 1:
                    # proj(c-1) spread across filler slots instead of one
                    # mid-conv burst: evens out the ACT/DVE staging load
                    fills = fills + proj_units(c - 1)
                if c + 2 < CROPS:
                    xload(c + 2)
                conv_phase(c + 1, fills)
                attn_tail(c)
            else:
                epilogue(c)

    nc.compile()
    return nc


def _host_inputs(x, wq, bq, wk, bk, wv, bv, w_proj):
    """Per-core input maps; conv output channels ordered [q, k, v]."""
    # Shared across cores: the 2-D Winograd F(2x2,3x3) input transform
    # U = B^T d B per 4x4 patch (stride 2), computed on the host from the
    # bf16-cast padded image. SBUF layout [128, jH, ci, jW, t2*16+t1].
    xf = np.asarray(x, dtype=np.float32).reshape(CROPS, C, H, W)
    xpad = np.zeros((CROPS, C, H + 2, W + 2), np.float32)
    xpad[:, :, 1:1 + H, 1:1 + W] = xf.astype(_BF16).astype(np.float32)
    win = np.lib.stride_tricks.sliding_window_view(xpad, (NJ, NJ), axis=(2, 3))
    win = win[:, :, ::2, ::2]                     # [n, c, 16, 16, a2, a1]
    u = np.einsum('ja,kb,nctsab->ncjkts', _BT, _BT, win, optimize=True)
    u = u.reshape(CROPS, CIT, 128, NJ, NJ, TS).transpose(0, 2, 3, 1, 4, 5)
    u = np.ascontiguousarray(u).reshape(CROPS, 128, NJ * GCOLS).astype(np.float16)

    in_maps = []
    for h in range(NCORES):
        sl = slice(h * HD, (h + 1) * HD)
        zpad = np.zeros((16,) + wq.shape[1:], wq.dtype)
        w_cat = np.concatenate(
            [wq[sl], wv[sl][:48], wk[sl], wv[sl][48:], zpad], axis=0)  # [256,...]
        # 2-D G-transform: [CIT, 128, jH, jW, co] = [5, 128, 4096]
        wg = np.einsum('jd,ke,ocde->cjko', _G, _G, w_cat.astype(np.float64),
                       optimize=True)
        wg = wg.reshape(CIT, 128, NJ, NJ, 256).reshape(CIT, 128, NJ * NJ * 256)
        b_cat = np.concatenate(
            [bq[sl], bv[sl][:48], bk[sl], bv[sl][48:],
             np.zeros(16, bq.dtype)]).reshape(256, 1)
        wpT = np.ascontiguousarray(w_proj[:, sl].T)  # [80, 640]
        in_maps.append({
            "u": u,
            "wg": np.ascontiguousarray(wg).astype(np.float16),
            "bqkv": b_cat.astype(np.float32),
            "wproj": wpT.astype(_BF16),
        })
    return in_maps


def _host_reduce(results, b_proj):
    acc = np.zeros((CROPS, C, N), np.float32)
    for r in results:
        acc += r["out"].astype(np.float32) / r["rsum"]
    # un-permute n' = slab(oW,oH)*256 + t2*16 + t1 back to n = y*32 + x
    idx = np.empty(N, np.int64)
    pos = 0
    for ow in range(2):
        for oh in range(2):
            for t2 in range(TT):
                for t1 in range(TT):
                    idx[pos] = (2 * t2 + oh) * W + (2 * t1 + ow)
                    pos += 1
    out = np.empty_like(acc)
    out[:, :, idx] = acc
    acc = out
    o = acc.reshape(BS, MC, C, N).transpose(0, 3, 1, 2)  # [b, n, m, dout]
    o = o + b_proj[None, None, None, :].astype(np.float32)
    return np.ascontiguousarray(o.reshape(BS, N, MC * C), dtype=np.float32)


_NC_CACHE = {}


def kernel(x, wq, bq, wk, bk, wv, bv, w_proj, b_proj, _run_kwargs=None):
    from concourse.bass_utils import run_bass_kernel_spmd

    if "nc" not in _NC_CACHE:
        _NC_CACHE["nc"] = _build_graph()
    nc = _NC_CACHE["nc"]
    in_maps = _host_inputs(x, wq, bq, wk, bk, wv, bv, w_proj)
    res = run_bass_kernel_spmd(nc, in_maps, core_ids=list(range(NCORES)),
                               **(_run_kwargs or {}))
    out = _host_reduce(res.results, np.asarray(b_proj))
    if _run_kwargs:
        _NC_CACHE["last_result"] = res
    return out
